# revision 1
# baseline (speedup 1.0000x reference)
"""Trainium2 Bass kernel for nn_CrossCorrelation.

Reference computation (per batch b of 8, c=32 channels of 128x128):
  xs = standardize(x)  (zero mean, unit "energy": / (unbiased_std * sqrt(n)))
  Xf = fft2(xs); for all ordered channel pairs (i, j>=i):
  cc = real(ifft2(Xf_i * conj(Xf_j))), rolled by (10,10), windowed to 21x21.

Device algorithm (one batch per NeuronCore, 8 cores):
  - FFTs as matmuls with DFT matrices (f32r full-rate fp32 path).
  - rfft along y (u in 0..64, Hermitian weights folded into the inverse).
  - Mean subtraction == zeroing the DC bin of the spectrum.
  - Per-channel scale folded into the spectrum planes.
  - Cross spectra via 3-mult Gauss complex product; the 2 post-adds are
    folded into the inverse-transform matmul stationaries (PSUM accum).
  - Inverse transform: D = S@m (contract v), PE transpose, out = G@D^T
    (contract u), 21x21 window baked into S/G (roll included).
"""

import os
import numpy as np

H = W = 128
C = 32
B = 8
NPIX = H * W
MAX_S = 10
S = 2 * MAX_S + 1  # 21
NPAIR = C * (C + 1) // 2  # 528
STD_EPS = 1e-9

UPAD = 66  # per-channel u-stride (65 used + 1 pad for bf16 4B alignment)
NU = 65    # rfft bins along y


def _host_constants():
    import ml_dtypes

    k = np.arange(H)
    F = np.exp(-2j * np.pi * np.outer(k, k) / H)  # symmetric DFT matrix
    Fr = np.ascontiguousarray(F.real, np.float32)
    Fi = np.ascontiguousarray(F.imag, np.float32)

    # Stage A moving operand: [Fr | Fi] split into a bf16 hi/lo pair
    frfi = np.concatenate([Fr, Fi], axis=1).astype(np.float32)  # (128, 256)
    ff_hi = frfi.astype(ml_dtypes.bfloat16)
    ff_lo = (frfi - ff_hi.astype(np.float32)).astype(ml_dtypes.bfloat16)
    ffs = np.concatenate([ff_hi, ff_lo], axis=1)  # (128, 512) bf16
    # Stage B stationaries (bf16): Fr, Fi, -Fi
    fmats = np.concatenate([Fr, Fi, -Fi], axis=1).astype(ml_dtypes.bfloat16)  # (128, 384)

    # Inverse-side matrices. Output row s corresponds to shift (s - 10) mod 128.
    sy = (np.arange(S) - MAX_S) % H
    u = np.arange(NU)
    Gy = np.exp(2j * np.pi * np.outer(sy, u) / H)  # (21, 65)
    w_u = np.ones(NU)
    w_u[1:64] = 2.0  # Hermitian fold weights for rfft-y
    Gyw = Gy * w_u
    Gx = np.exp(2j * np.pi * np.outer(sy, np.arange(W)) / W) / NPIX  # (21, 128)

    Gxr = Gx.real.astype(np.float32)
    Gxi = Gx.imag.astype(np.float32)
    S1 = np.concatenate([Gxr, Gxi], axis=0)  # (42, 128)
    S2 = np.concatenate([-Gxi, Gxr], axis=0)
    S12 = S1 - S2
    # pad each stationary to 64 output rows (rows 42..63 produce zeros) so
    # two 7-pair groups stack into one PSUM bank at partition offsets 0/64
    pad = np.zeros((22, 128), np.float32)
    smats = np.concatenate(
        [np.concatenate([S, pad], axis=0).T for S in (S1, S12, S2)],
        axis=1)  # (128, 192)
    smats = smats.astype(ml_dtypes.bfloat16)

    Gywr = Gyw.real.astype(np.float32)
    Gywi = Gyw.imag.astype(np.float32)
    gys = np.concatenate([Gywr.T, (-Gywi).T], axis=1)  # (65, 42)
    gys = gys.astype(ml_dtypes.bfloat16)

    id128 = np.eye(128, dtype=ml_dtypes.bfloat16)
    ones_col = np.ones((128, 1), ml_dtypes.bfloat16)
    ones_row = np.ones((1, 128), ml_dtypes.bfloat16)

    return dict(
        ffs=ffs, fmats=fmats, smats=smats, gys=gys, id128=id128,
        ones_col=ones_col, ones_row=ones_row,
    )


def build_nc():
    """Build the single-core Bass program (SPMD across 8 cores)."""
    import concourse.bass as bass
    import concourse.mybir as mybir
    import concourse.tile as tile
    from concourse import bacc
    from contextlib import ExitStack

    f32 = mybir.dt.float32
    f32r = mybir.dt.float32r
    bf16 = mybir.dt.bfloat16
    AF = mybir.ActivationFunctionType
    ALU = mybir.AluOpType

    nc = bacc.Bacc("TRN2", target_bir_lowering=False, debug=False)

    x_d = nc.dram_tensor("x", [C, H, W], f32, kind="ExternalInput").ap()
    ffs_d = nc.dram_tensor("ffs", [128, 512], bf16, kind="ExternalInput").ap()
    onesc_d = nc.dram_tensor("ones_col", [128, 1], bf16, kind="ExternalInput").ap()
    onesr_d = nc.dram_tensor("ones_row", [1, 128], bf16, kind="ExternalInput").ap()
    fmats_d = nc.dram_tensor("fmats", [128, 384], bf16, kind="ExternalInput").ap()
    smats_d = nc.dram_tensor("smats", [128, 192], bf16, kind="ExternalInput").ap()
    gys_d = nc.dram_tensor("gys", [65, 42], bf16, kind="ExternalInput").ap()
    id128_d = nc.dram_tensor("id128", [128, 128], bf16, kind="ExternalInput").ap()
    out_d = nc.dram_tensor("out", [NPAIR, S, S], f32, kind="ExternalOutput").ap()

    with tile.TileContext(nc) as tc, ExitStack() as ctx:
        cpool = ctx.enter_context(tc.tile_pool(name="consts", bufs=1))
        spool = ctx.enter_context(tc.tile_pool(name="work", bufs=1))

        # ---- constants + input loads ----
        fmats = cpool.tile([128, 384], bf16, tag="fmats")
        nc.sync.dma_start(fmats[:, :], fmats_d)
        smats = cpool.tile([128, 192], bf16, tag="smats")
        nc.sync.dma_start(smats[:, :], smats_d)
        gys = cpool.tile([65, 42], bf16, tag="gys")
        nc.sync.dma_start(gys[:, :], gys_d)
        id128 = cpool.tile([128, 128], bf16, tag="id128")
        nc.sync.dma_start(id128[:, :], id128_d)
        Xt = spool.tile([128, C, W], f32, tag="X")  # partition=y, free=(c, x)
        for k in range(0, C, 8):
            nc.sync.dma_start(Xt[:, k:k + 8, :],
                              x_d[k:k + 8].transpose([1, 0, 2]))
        X = Xt
        ffs = cpool.tile([128, 512], bf16, tag="ffs")
        nc.sync.dma_start(ffs[:, :], ffs_d)
        ones_col = cpool.tile([128, 1], bf16, tag="ones_col")
        nc.sync.dma_start(ones_col[:, :], onesc_d)
        ones_row = cpool.tile([1, 128], bf16, tag="ones_row")
        nc.sync.dma_start(ones_row[:, :], onesr_d)

        Fr = fmats[:, 0:128]
        Fi = fmats[:, 128:256]
        Fin = fmats[:, 256:384]

        # ---- persistent SBUF work tensors ----
        T_s = spool.tile([128, C, 2, UPAD], bf16, tag="T")     # (x, c, re/im, u)
        P1 = spool.tile([128, C, UPAD], bf16, tag="P1")       # (r+i)*s
        P2 = spool.tile([128, C, UPAD], bf16, tag="P2")       # i*s
        P3 = spool.tile([128, C, UPAD], bf16, tag="P3")       # (i-r)*s
        P4 = spool.tile([128, C, UPAD], bf16, tag="P4")       # r*s
        bc = spool.tile([128, 64], f32, tag="bc")             # bcast [s | -s]

        # zero the pad column (products read it; keep finite -> 0*0)
        for P in (P1, P2, P3, P4):
            nc.vector.memset(P[:, :, 65:66], 0.0)

        # =========================== phase 1 ===========================
        with tc.tile_pool(name="psA", bufs=2, space="PSUM") as psA, \
             tc.tile_pool(name="psB", bufs=2, space="PSUM") as psB, \
             tc.tile_pool(name="psS", bufs=1, space="PSUM") as psS:

            # ---- per-channel stats (chunked to overlap the DMA) ----
            sq = spool.tile([128, C, W], f32, tag="sq")
            red = spool.tile([128, 64], f32, tag="red")
            xh = spool.tile([128, C, W], bf16, tag="xh")
            xl = spool.tile([128, C, W], bf16, tag="xl")
            for k in range(0, C, 8):
                s = slice(k, k + 8)
                nc.scalar.activation(xh[:, s, :], X[:, s, :], AF.Copy)
                nc.vector.tensor_tensor(xl[:, s, :], X[:, s, :], xh[:, s, :],
                                        op=ALU.subtract)
                nc.scalar.activation(sq[:, s, :], X[:, s, :], AF.Square)
                nc.vector.tensor_reduce(
                    red[:, k:k + 8], X[:, s, :],
                    axis=mybir.AxisListType.X, op=ALU.add)
                nc.vector.tensor_reduce(
                    red[:, 32 + k:40 + k], sq[:, s, :],
                    axis=mybir.AxisListType.X, op=ALU.add)
            red_hi = spool.tile([128, 64], bf16, tag="red_hi")
            nc.scalar.activation(red_hi[:, :], red[:, :], AF.Copy)
            red_lo = spool.tile([128, 64], bf16, tag="red_lo")
            nc.vector.tensor_tensor(red_lo[:, :], red[:, :], red_hi[:, :],
                                    op=ALU.subtract)
            stats_ps = psS.tile([1, 64], f32, tag="stats")
            nc.tensor.matmul(stats_ps[:, :], ones_col[:, :], red_hi[:, :],
                             start=True, stop=False)
            nc.tensor.matmul(stats_ps[:, :], ones_col[:, :], red_lo[:, :],
                             start=False, stop=True)

            n = float(NPIX)
            ssq = spool.tile([1, 32], f32, tag="ssq")
            nc.scalar.activation(ssq[:, :], stats_ps[:, 0:32], AF.Square)
            qn = spool.tile([1, 32], f32, tag="qn")
            nc.vector.tensor_scalar_mul(qn[:, :], stats_ps[:, 32:64], 1.0 / (n - 1.0))
            ssqs = spool.tile([1, 32], f32, tag="ssqs")
            nc.vector.tensor_scalar_mul(ssqs[:, :], ssq[:, :], -1.0 / (n * (n - 1.0)))
            var = spool.tile([1, 32], f32, tag="var")
            nc.vector.tensor_tensor(var[:, :], ssqs[:, :], qn[:, :], op=ALU.add)
            mask = spool.tile([1, 32], f32, tag="mask")
            nc.vector.tensor_scalar(mask[:, :], var[:, :], STD_EPS * STD_EPS, None,
                                    op0=ALU.is_ge)
            tn = spool.tile([1, 32], f32, tag="tn")
            nc.vector.tensor_scalar(tn[:, :], var[:, :], 1e-30, n,
                                    op0=ALU.max, op1=ALU.mult)
            rcp = spool.tile([1, 32], f32, tag="rcp")
            nc.vector.reciprocal(rcp[:, :], tn[:, :])
            rs = spool.tile([1, 32], f32, tag="rs")
            nc.scalar.sqrt(rs[:, :], rcp[:, :])  # 1/(std*sqrt(n))
            sc2 = spool.tile([1, 64], f32, tag="sc2")
            nc.vector.tensor_tensor(sc2[:, 0:32], rs[:, :], mask[:, :], op=ALU.mult)
            nc.vector.tensor_scalar_mul(sc2[:, 32:64], sc2[:, 0:32], -1.0)
            sc2h = spool.tile([1, 64], bf16, tag="sc2h")
            nc.scalar.activation(sc2h[:, :], sc2[:, :], AF.Copy)
            sc2l = spool.tile([1, 64], bf16, tag="sc2l")
            nc.vector.tensor_tensor(sc2l[:, :], sc2[:, :], sc2h[:, :],
                                    op=ALU.subtract)
            bc_ps = psS.tile([128, 64], f32, tag="bcps")
            nc.tensor.matmul(bc_ps[:, :], ones_row[:, :], sc2h[:, :],
                             start=True, stop=False)
            nc.tensor.matmul(bc_ps[:, :], ones_row[:, :], sc2l[:, :],
                             start=False, stop=True)
            nc.scalar.copy(bc[:, :], bc_ps[:, :])

            # ---- stage A: y-FFT per channel: T^T = x_c^T @ [Fr|Fi] ----
            # split-bf16: x = xh + xl; T = xh@ff_hi + xh@ff_lo + xl@ff_hi
            for c in range(C):
                pa = psA.tile([128, 2, 128], f32, tag="pa")
                pav = pa[:, :, :].rearrange("p a b -> p (a b)")
                nc.tensor.matmul(pav, xh[:, c, :], ffs[:, 0:256],
                                 start=True, stop=False)
                nc.tensor.matmul(pav, xh[:, c, :], ffs[:, 256:512],
                                 start=False, stop=False)
                nc.tensor.matmul(pav, xl[:, c, :], ffs[:, 0:256],
                                 start=False, stop=True)
                nc.scalar.activation(T_s[:, c, :, 0:65], pa[:, :, 0:65], AF.Copy,
                                     scale=bc[:, c:c + 1])

            # ---- stage B: x-FFT + scaled Gauss planes ----
            for g in range(0, C, 7):
                w = min(7, C - g)
                br = psB.tile([128, 7, 65], f32, tag="br")
                bi = psB.tile([128, 7, 65], f32, tag="bi")
                TrT = T_s[:, g:g + w, 0, 0:65]
                TiT = T_s[:, g:g + w, 1, 0:65]
                nc.tensor.matmul(br[:, :w, :], Fr, TrT, start=True, stop=False)
                nc.tensor.matmul(br[:, :w, :], Fin, TiT, start=False, stop=True)
                nc.tensor.matmul(bi[:, :w, :], Fi, TrT, start=True, stop=False)
                nc.tensor.matmul(bi[:, :w, :], Fr, TiT, start=False, stop=True)
                # zero each channel's DC bin [v=0,u=0] == mean subtraction
                nc.vector.memset(br[0:1, 0:w, 0:1], 0.0)
                nc.vector.memset(bi[0:1, 0:w, 0:1], 0.0)
                gs = slice(g, g + w)
                nc.scalar.activation(P4[:, gs, 0:65], br[:, 0:w, :], AF.Copy)
                nc.scalar.activation(P2[:, gs, 0:65], bi[:, 0:w, :], AF.Copy)
                nc.vector.tensor_tensor(P1[:, gs, 0:65], P4[:, gs, 0:65],
                                        P2[:, gs, 0:65], op=ALU.add)
                nc.vector.tensor_tensor(P3[:, gs, 0:65], P2[:, gs, 0:65],
                                        P4[:, gs, 0:65], op=ALU.subtract)


        # =========================== phase 2 ===========================
        BUFS = [int(v) for v in os.environ.get(
            "K_BUFS", "4,12,4,3,2,3").split(",")]
        with tc.tile_pool(name="mpool", bufs=BUFS[0]) as mpool, \
             tc.tile_pool(name="dspool", bufs=BUFS[1]) as dspool, \
             tc.tile_pool(name="dtpool", bufs=BUFS[2]) as dtpool, \
             tc.tile_pool(name="psD", bufs=BUFS[3], space="PSUM") as psD, \
             tc.tile_pool(name="psDT", bufs=BUFS[4], space="PSUM") as psDT, \
             tc.tile_pool(name="psO", bufs=BUFS[5], space="PSUM") as psO:

            # A "subgroup" is <=7 pairs of one i-block. Two subgroups of
            # equal width stack into one D-PSUM bank (partition offsets 0/64;
            # S-matrices are padded to 64 rows so rows 42..63 are zeros).
            out_copy_flip = [0]

            def emit_dt_banks(ds, subA, subB):
                (sA, w, pA) = subA
                nhalf = 2 if subB is not None else 1
                pB = subB[2] if subB is not None else None
                dt_ps = psDT.tile([65, 8, 2, 64], bf16, tag="dt")
                for t in range(w):
                    if nhalf == 2:
                        nc.tensor.transpose(dt_ps[:, t, :, :],
                                            ds[:, t, :], id128[:, :])
                    else:
                        nc.tensor.transpose(dt_ps[:, t, 0, :],
                                            ds[0:64, t, :],
                                            id128[0:64, 0:64])
                dt_s = dtpool.tile([65, 8, 2, 64], bf16, tag="dts")
                if out_copy_flip[0] % 2 == 0:
                    nc.scalar.activation(dt_s[:, 0:w, 0:nhalf, 0:42],
                                         dt_ps[:, 0:w, 0:nhalf, 0:42],
                                         AF.Copy)
                else:
                    nc.vector.tensor_copy(dt_s[:, 0:w, 0:nhalf, 0:42],
                                          dt_ps[:, 0:w, 0:nhalf, 0:42])
                op_ps = psO.tile([21, 8, 2, 21], f32, tag="ops")
                ov = op_ps[:, 0:w, 0:nhalf, :]
                nc.tensor.matmul(ov, gys[:, 0:21],
                                 dt_s[:, 0:w, 0:nhalf, 0:21],
                                 start=True, stop=False)
                nc.tensor.matmul(ov, gys[:, 21:42],
                                 dt_s[:, 0:w, 0:nhalf, 21:42],
                                 start=False, stop=True)
                out_s = dtpool.tile([21, 8, 2, 21], f32, tag="outs")
                oc = out_s[:, 0:w, 0:nhalf, :]
                if out_copy_flip[0] % 2 == 1:
                    nc.vector.tensor_copy(oc, ov)
                else:
                    nc.scalar.activation(oc, ov, AF.Copy)
                out_copy_flip[0] += 1
                nc.sync.dma_start(
                    out_d[pA:pA + w, :, :].transpose([1, 0, 2]),
                    out_s[:, 0:w, 0, :])
                if nhalf == 2:
                    nc.sync.dma_start(
                        out_d[pB:pB + w, :, :].transpose([1, 0, 2]),
                        out_s[:, 0:w, 1, :])

            def emit_bank(mA, subA, mB, subB):
                (sA, w, pA) = subA
                dps = psD.tile([128, 7, 65], f32, tag="d")
                for t in range(3):
                    st = smats[:, 64 * t:64 * t + 64]
                    nc.tensor.matmul(dps[0:64, 0:w, :], st,
                                     mA[t][:, sA:sA + w, 0:65],
                                     start=(t == 0), stop=(t == 2))
                if subB is not None:
                    (sB, wB, pB) = subB
                    for t in range(3):
                        st = smats[:, 64 * t:64 * t + 64]
                        nc.tensor.matmul(dps[64:128, 0:w, :], st,
                                         mB[t][:, sB:sB + w, 0:65],
                                         start=(t == 0), stop=(t == 2),
                                         tile_position=(0, 64))
                ds = dspool.tile([128, 7, 65], bf16, tag="ds")
                if subB is not None:
                    nc.scalar.activation(ds[:, 0:w, :], dps[:, 0:w, :], AF.Copy)
                else:
                    nc.scalar.activation(ds[0:64, 0:w, :], dps[0:64, 0:w, :],
                                         AF.Copy)
                emit_dt_banks(ds, subA, subB)

            pend7 = None
            pair_base = 0
            for i in range(C):
                npairs = C - i
                m1 = mpool.tile([128, C, UPAD], bf16, tag="m1")
                m2 = mpool.tile([128, C, UPAD], bf16, tag="m2")
                m3 = mpool.tile([128, C, UPAD], bf16, tag="m3")
                bshape = [128, npairs, UPAD]
                nc.vector.tensor_tensor(m1[:, 0:npairs, :],
                                        P1[:, i:i + 1, :].broadcast_to(bshape),
                                        P4[:, i:, :], op=ALU.mult)
                nc.vector.tensor_tensor(m2[:, 0:npairs, :],
                                        P2[:, i:i + 1, :].broadcast_to(bshape),
                                        P3[:, i:, :], op=ALU.mult)
                nc.vector.tensor_tensor(m3[:, 0:npairs, :],
                                        P3[:, i:i + 1, :].broadcast_to(bshape),
                                        P2[:, i:, :], op=ALU.mult)
                mt = (m1, m2, m3)
                for s0 in range(0, npairs, 7):
                    w = min(7, npairs - s0)
                    sub = (s0, w, pair_base + s0)
                    if w == 7:
                        if pend7 is None:
                            pend7 = (mt, sub)
                        else:
                            (mA, subA) = pend7
                            pend7 = None
                            emit_bank(mA, subA, mt, sub)
                    else:
                        emit_bank(mt, sub, None, None)
                pair_base += npairs
            if pend7 is not None:
                (mA, subA) = pend7
                emit_bank(mA, subA, None, None)

    nc.compile()
    return nc


_CACHE = {}


def _get_nc():
    if "nc" not in _CACHE:
        _CACHE["nc"] = build_nc()
    return _CACHE["nc"]


TRACE = False  # test harness can flip this to capture an NTFF profile


def kernel(x: np.ndarray) -> np.ndarray:
    from concourse.bass_utils import run_bass_kernel_spmd

    assert x.shape == (B, C, H, W) and x.dtype == np.float32
    nc = _get_nc()
    consts = _host_constants()
    in_maps = []
    for b in range(B):
        m = {"x": np.ascontiguousarray(x[b])}
        m.update(consts)
        in_maps.append(m)
    res = run_bass_kernel_spmd(nc, in_maps, core_ids=list(range(B)), trace=TRACE)
    _CACHE["last_results"] = res
    out = np.stack([r["out"] for r in res.results]).astype(np.float32)
    return out



# revision 4
# speedup vs baseline: 1.1154x; 1.1154x over previous
"""Trainium2 Bass kernel for nn_CrossCorrelation.

Reference computation (per batch b of 8, c=32 channels of 128x128):
  xs = standardize(x)  (zero mean, unit "energy": / (unbiased_std * sqrt(n)))
  Xf = fft2(xs); for all ordered channel pairs (i, j>=i):
  cc = real(ifft2(Xf_i * conj(Xf_j))), rolled by (10,10), windowed to 21x21.

Device algorithm (one batch per NeuronCore, 8 cores):
  - FFTs as matmuls with DFT matrices (bf16 hi/lo split for precision).
  - rfft along y (u in 0..64, Hermitian weights folded into the inverse).
  - Mean subtraction == zeroing the DC bin of the spectrum.
  - Per-channel scale folded into the spectrum planes.
  - Cross spectra via 3-mult Gauss complex product (DVE/GpSimd); the 2
    post-adds are folded into the inverse-transform matmul operands.
  - Inverse x-transform with the PRODUCT as the matmul stationary and the
    42-row Gx-combo matrix as moving: D^T = m^T @ S^T lands in PSUM with
    u already on partitions -- no PE transpose needed.
  - 12 pairs share one PSUM bank (12*42*4B = 2016B); one drain per bank.
  - Inverse y-transform: 2 matmuls per 12-pair group (Gyw stationary),
    21x21 window + roll baked into S/G matrices.
  - Output staged in SBUF, written with 4 large DMAs.
"""

import os
import numpy as np

H = W = 128
C = 32
B = 8
NPIX = H * W
MAX_S = 10
S = 2 * MAX_S + 1  # 21
NPAIR = C * (C + 1) // 2  # 528
STD_EPS = 1e-9

UPAD = 66  # per-channel u-stride (65 used + 1 pad for 4B alignment)
NU = 65    # rfft bins along y

# const layout (columns in the merged [128, NCONST] bf16 tensor)
OFF_FFS = 0            # [128, 512]  [Fr|Fi] hi, then lo
OFF_FMATS = 512        # [128, 384]  Fr, Fi, -Fi
OFF_SMT = 896          # [128, 126]  S1^T | S12^T | S2^T  (42 cols each)
OFF_GYS = 1022         # [65, 42]    [Gywr^T | -Gywi^T]
OFF_ONEC = 1064        # [128, 1]    ones column
OFF_ONER = 1065        # [1, 128]    ones row (partition 0)
NCONST = 1193

PAIRS_PER_BANK = 12    # 12 * 42 * 4B = 2016B <= 2KB PSUM bank
BANKS_PER_O = 2        # 24 pairs per G-output PSUM bank
DMA_CHUNK = 132        # pairs per output DMA (528 = 4 * 132)


def _host_constants():
    import ml_dtypes

    k = np.arange(H)
    F = np.exp(-2j * np.pi * np.outer(k, k) / H)  # symmetric DFT matrix
    Fr = np.ascontiguousarray(F.real, np.float32)
    Fi = np.ascontiguousarray(F.imag, np.float32)

    # Stage A moving operand: [Fr | Fi] split into a bf16 hi/lo pair
    frfi = np.concatenate([Fr, Fi], axis=1).astype(np.float32)  # (128, 256)
    ff_hi = frfi.astype(ml_dtypes.bfloat16)
    ff_lo = (frfi - ff_hi.astype(np.float32)).astype(ml_dtypes.bfloat16)
    ffs = np.concatenate([ff_hi, ff_lo], axis=1).astype(np.float32)  # (128, 512)
    # Stage B stationaries: Fr, Fi, -Fi
    fmats = np.concatenate([Fr, Fi, -Fi], axis=1)  # (128, 384)

    # Inverse-side matrices. Output row s corresponds to shift (s - 10) mod 128.
    sy = (np.arange(S) - MAX_S) % H
    u = np.arange(NU)
    Gy = np.exp(2j * np.pi * np.outer(sy, u) / H)  # (21, 65)
    w_u = np.ones(NU)
    w_u[1:64] = 2.0  # Hermitian fold weights for rfft-y
    Gyw = Gy * w_u
    Gx = np.exp(2j * np.pi * np.outer(sy, np.arange(W)) / W) / NPIX  # (21, 128)

    Gxr = Gx.real.astype(np.float32)
    Gxi = Gx.imag.astype(np.float32)
    S1 = np.concatenate([Gxr, Gxi], axis=0)   # (42, 128)
    S2 = np.concatenate([-Gxi, Gxr], axis=0)  # (42, 128)
    S12 = S1 - S2
    # D-step MOVING operands: transposed, no padding (42 cols each)
    smt = np.concatenate([S1.T, S12.T, S2.T], axis=1)  # (128, 126)

    Gywr = Gyw.real.astype(np.float32)
    Gywi = Gyw.imag.astype(np.float32)
    gys = np.concatenate([Gywr.T, (-Gywi).T], axis=1)  # (65, 42)

    consts = np.zeros((128, NCONST), np.float32)
    consts[:, OFF_FFS:OFF_FFS + 512] = ffs
    consts[:, OFF_FMATS:OFF_FMATS + 384] = fmats
    consts[:, OFF_SMT:OFF_SMT + 126] = smt
    consts[0:65, OFF_GYS:OFF_GYS + 42] = gys
    consts[:, OFF_ONEC] = 1.0
    consts[0, OFF_ONER:OFF_ONER + 128] = 1.0
    return dict(consts=consts.astype(ml_dtypes.bfloat16))


def build_nc():
    """Build the single-core Bass program (SPMD across 8 cores)."""
    import concourse.bass as bass
    import concourse.mybir as mybir
    import concourse.tile as tile
    from concourse import bacc
    from contextlib import ExitStack

    f32 = mybir.dt.float32
    bf16 = mybir.dt.bfloat16
    AF = mybir.ActivationFunctionType
    ALU = mybir.AluOpType

    nc = bacc.Bacc("TRN2", target_bir_lowering=False, debug=False)

    x_d = nc.dram_tensor("x", [C, H, W], f32, kind="ExternalInput").ap()
    consts_d = nc.dram_tensor("consts", [128, NCONST], bf16,
                              kind="ExternalInput").ap()
    out_d = nc.dram_tensor("out", [NPAIR, S, S], f32, kind="ExternalOutput").ap()

    with tile.TileContext(nc) as tc, ExitStack() as ctx:
        cpool = ctx.enter_context(tc.tile_pool(name="consts", bufs=1))
        spool = ctx.enter_context(tc.tile_pool(name="work", bufs=1))

        # ---- constants (single DMA) + input loads ----
        cst = cpool.tile([128, NCONST], bf16, tag="consts")
        nc.sync.dma_start(cst[:, :], consts_d)
        ffs = cst[:, OFF_FFS:OFF_FFS + 512]
        Fr = cst[:, OFF_FMATS:OFF_FMATS + 128]
        Fi = cst[:, OFF_FMATS + 128:OFF_FMATS + 256]
        Fin = cst[:, OFF_FMATS + 256:OFF_FMATS + 384]
        smt = cst[:, OFF_SMT:OFF_SMT + 126]
        gys = cst[0:65, OFF_GYS:OFF_GYS + 42]
        ones_col = cst[:, OFF_ONEC:OFF_ONEC + 1]
        ones_row = cst[0:1, OFF_ONER:OFF_ONER + 128]

        Xt = spool.tile([128, C, W], f32, tag="X")  # partition=y, free=(c, x)
        for k in range(0, C, 8):
            nc.sync.dma_start(Xt[:, k:k + 8, :],
                              x_d[k:k + 8].transpose([1, 0, 2]))
        X = Xt

        # ---- persistent SBUF work tensors ----
        T_s = spool.tile([128, C, 2, UPAD], bf16, tag="T")    # (x, c, re/im, u)
        P1 = spool.tile([128, C, UPAD], bf16, tag="P1")       # (r+i)*s
        P2 = spool.tile([128, C, UPAD], bf16, tag="P2")       # i*s
        P3 = spool.tile([128, C, UPAD], bf16, tag="P3")       # (i-r)*s
        P4 = spool.tile([128, C, UPAD], bf16, tag="P4")       # r*s
        bc = spool.tile([128, 32], f32, tag="bc")             # bcast scale
        stage = spool.tile([S, NPAIR, S], f32, tag="stage")   # out staging

        # =========================== phase 1 ===========================
        with tc.tile_pool(name="psA", bufs=2, space="PSUM") as psA, \
             tc.tile_pool(name="psB", bufs=2, space="PSUM") as psB, \
             tc.tile_pool(name="psS", bufs=1, space="PSUM") as psS:

            # ---- per-channel stats (chunked to overlap the DMA) ----
            sq = spool.tile([128, C, W], f32, tag="sq")
            red = spool.tile([128, 64], f32, tag="red")
            xh = spool.tile([128, C, W], bf16, tag="xh")
            xl = spool.tile([128, C, W], bf16, tag="xl")
            for k in range(0, C, 8):
                s = slice(k, k + 8)
                nc.scalar.activation(xh[:, s, :], X[:, s, :], AF.Copy)
                nc.gpsimd.tensor_tensor(xl[:, s, :], X[:, s, :], xh[:, s, :],
                                        op=ALU.subtract)
                nc.scalar.activation(sq[:, s, :], X[:, s, :], AF.Square)
                nc.vector.tensor_reduce(
                    red[:, k:k + 8], X[:, s, :],
                    axis=mybir.AxisListType.X, op=ALU.add)
                nc.vector.tensor_reduce(
                    red[:, 32 + k:40 + k], sq[:, s, :],
                    axis=mybir.AxisListType.X, op=ALU.add)
            red_hi = spool.tile([128, 64], bf16, tag="red_hi")
            nc.scalar.activation(red_hi[:, :], red[:, :], AF.Copy)
            red_lo = spool.tile([128, 64], bf16, tag="red_lo")
            nc.vector.tensor_tensor(red_lo[:, :], red[:, :], red_hi[:, :],
                                    op=ALU.subtract)
            stats_ps = psS.tile([1, 64], f32, tag="stats")
            nc.tensor.matmul(stats_ps[:, :], ones_col, red_hi[:, :],
                             start=True, stop=False)
            nc.tensor.matmul(stats_ps[:, :], ones_col, red_lo[:, :],
                             start=False, stop=True)

            n = float(NPIX)
            ssq = spool.tile([1, 32], f32, tag="ssq")
            nc.scalar.activation(ssq[:, :], stats_ps[:, 0:32], AF.Square)
            qn = spool.tile([1, 32], f32, tag="qn")
            nc.vector.tensor_scalar_mul(qn[:, :], stats_ps[:, 32:64],
                                        1.0 / (n - 1.0))
            ssqs = spool.tile([1, 32], f32, tag="ssqs")
            nc.vector.tensor_scalar_mul(ssqs[:, :], ssq[:, :],
                                        -1.0 / (n * (n - 1.0)))
            var = spool.tile([1, 32], f32, tag="var")
            nc.vector.tensor_tensor(var[:, :], ssqs[:, :], qn[:, :], op=ALU.add)
            mask = spool.tile([1, 32], f32, tag="mask")
            nc.vector.tensor_scalar(mask[:, :], var[:, :], STD_EPS * STD_EPS,
                                    None, op0=ALU.is_ge)
            tn = spool.tile([1, 32], f32, tag="tn")
            nc.vector.tensor_scalar(tn[:, :], var[:, :], 1e-30, n,
                                    op0=ALU.max, op1=ALU.mult)
            rcp = spool.tile([1, 32], f32, tag="rcp")
            nc.vector.reciprocal(rcp[:, :], tn[:, :])
            rs = spool.tile([1, 32], f32, tag="rs")
            nc.scalar.sqrt(rs[:, :], rcp[:, :])  # 1/(std*sqrt(n))
            sc2 = spool.tile([1, 32], f32, tag="sc2")
            nc.vector.tensor_tensor(sc2[:, :], rs[:, :], mask[:, :],
                                    op=ALU.mult)
            sc2h = spool.tile([1, 32], bf16, tag="sc2h")
            nc.scalar.activation(sc2h[:, :], sc2[:, :], AF.Copy)
            sc2l = spool.tile([1, 32], bf16, tag="sc2l")
            nc.vector.tensor_tensor(sc2l[:, :], sc2[:, :], sc2h[:, :],
                                    op=ALU.subtract)
            bc_ps = psS.tile([128, 32], f32, tag="bcps")
            nc.tensor.matmul(bc_ps[:, :], ones_row, sc2h[:, :],
                             start=True, stop=False)
            nc.tensor.matmul(bc_ps[:, :], ones_row, sc2l[:, :],
                             start=False, stop=True)
            nc.scalar.copy(bc[:, :], bc_ps[:, :])

            # ---- stage A: y-FFT per channel: T^T = x_c^T @ [Fr|Fi] ----
            # split-bf16: x = xh + xl; T = xh@ff_hi + xh@ff_lo + xl@ff_hi
            for c in range(C):
                pa = psA.tile([128, 2, 128], f32, tag="pa")
                pav = pa[:, :, :].rearrange("p a b -> p (a b)")
                nc.tensor.matmul(pav, xh[:, c, :], ffs[:, 0:256],
                                 start=True, stop=False)
                nc.tensor.matmul(pav, xh[:, c, :], ffs[:, 256:512],
                                 start=False, stop=False)
                nc.tensor.matmul(pav, xl[:, c, :], ffs[:, 0:256],
                                 start=False, stop=True)
                nc.scalar.activation(T_s[:, c, :, 0:65], pa[:, :, 0:65],
                                     AF.Copy, scale=bc[:, c:c + 1])

            # ---- stage B: x-FFT + scaled Gauss planes ----
            for g in range(0, C, 7):
                w = min(7, C - g)
                br = psB.tile([128, 7, 65], f32, tag="br")
                bi = psB.tile([128, 7, 65], f32, tag="bi")
                TrT = T_s[:, g:g + w, 0, 0:65]
                TiT = T_s[:, g:g + w, 1, 0:65]
                nc.tensor.matmul(br[:, :w, :], Fr, TrT, start=True, stop=False)
                nc.tensor.matmul(br[:, :w, :], Fin, TiT, start=False, stop=True)
                nc.tensor.matmul(bi[:, :w, :], Fi, TrT, start=True, stop=False)
                nc.tensor.matmul(bi[:, :w, :], Fr, TiT, start=False, stop=True)
                # zero each channel's DC bin [v=0,u=0] == mean subtraction
                nc.vector.memset(br[0:1, 0:w, 0:1], 0.0)
                nc.vector.memset(bi[0:1, 0:w, 0:1], 0.0)
                gs = slice(g, g + w)
                nc.scalar.activation(P4[:, gs, 0:65], br[:, 0:w, :], AF.Copy)
                nc.scalar.activation(P2[:, gs, 0:65], bi[:, 0:w, :], AF.Copy)
                nc.vector.tensor_tensor(P1[:, gs, 0:65], P4[:, gs, 0:65],
                                        P2[:, gs, 0:65], op=ALU.add)
                nc.vector.tensor_tensor(P3[:, gs, 0:65], P2[:, gs, 0:65],
                                        P4[:, gs, 0:65], op=ALU.subtract)

        # =========================== phase 2 ===========================
        BUFS = [int(v) for v in os.environ.get("K_BUFS", "4,3,3,3").split(",")]
        # number of trailing i-blocks whose products go to GpSimd
        POOL_I = int(os.environ.get("K_POOL_I", "0"))

        with tc.tile_pool(name="mpool", bufs=BUFS[0]) as mpool, \
             tc.tile_pool(name="dtpool", bufs=BUFS[1]) as dtpool, \
             tc.tile_pool(name="psD", bufs=BUFS[2], space="PSUM") as psD, \
             tc.tile_pool(name="psO", bufs=BUFS[3], space="PSUM") as psO:

            mtiles = {}

            def emit_products(i):
                npairs = C - i
                m1 = mpool.tile([128, C, UPAD], bf16, tag="m1")
                m2 = mpool.tile([128, C, UPAD], bf16, tag="m2")
                m3 = mpool.tile([128, C, UPAD], bf16, tag="m3")
                bshape = [128, npairs, 65]
                eng = nc.gpsimd if i >= C - POOL_I else nc.vector
                eng.tensor_tensor(m1[:, 0:npairs, 0:65],
                                  P1[:, i:i + 1, 0:65].broadcast_to(bshape),
                                  P4[:, i:, 0:65], op=ALU.mult)
                eng.tensor_tensor(m2[:, 0:npairs, 0:65],
                                  P2[:, i:i + 1, 0:65].broadcast_to(bshape),
                                  P3[:, i:, 0:65], op=ALU.mult)
                eng.tensor_tensor(m3[:, 0:npairs, 0:65],
                                  P3[:, i:i + 1, 0:65].broadcast_to(bshape),
                                  P2[:, i:, 0:65], op=ALU.mult)
                mtiles[i] = (m1, m2, m3)

            # state for the pair pipeline
            state = dict(dps=None, slot=0, ops=None, ohalf=0, obase=0)

            def flush_bank():
                """Drain the current 12-pair D PSUM bank and run the G-step."""
                dps = state["dps"]
                nslot = state["slot"]
                if dps is None or nslot == 0:
                    return
                dts = dtpool.tile([65, PAIRS_PER_BANK, 42], bf16, tag="dts")
                nc.scalar.activation(dts[:, 0:nslot, :], dps[:, 0:nslot, :],
                                     AF.Copy)
                if state["ops"] is None:
                    state["ops"] = psO.tile(
                        [S, BANKS_PER_O * PAIRS_PER_BANK, S], f32,
                        name="ops", tag="ops")
                    state["ohalf"] = 0
                ops = state["ops"]
                o0 = state["ohalf"] * PAIRS_PER_BANK
                ov = ops[:, o0:o0 + nslot, :]
                nc.tensor.matmul(ov, gys[:, 0:21], dts[:, 0:nslot, 0:21],
                                 start=True, stop=False)
                nc.tensor.matmul(ov, gys[:, 21:42], dts[:, 0:nslot, 21:42],
                                 start=False, stop=True)
                state["ohalf"] += 1
                state["dps"] = None
                state["slot"] = 0
                if state["ohalf"] == BANKS_PER_O:
                    flush_obank(BANKS_PER_O * PAIRS_PER_BANK)

            def flush_obank(npairs_o):
                ops = state["ops"]
                if ops is None:
                    return
                ob = state["obase"]
                nc.vector.tensor_copy(stage[:, ob:ob + npairs_o, :],
                                      ops[:, 0:npairs_o, :])
                state["obase"] = ob + npairs_o
                state["ops"] = None
                state["ohalf"] = 0
                # emit output DMA when a chunk boundary is crossed
                b0 = (ob // DMA_CHUNK) * DMA_CHUNK
                if ob + npairs_o >= b0 + DMA_CHUNK:
                    nc.sync.dma_start(
                        out_d[b0:b0 + DMA_CHUNK].transpose([1, 0, 2]),
                        stage[:, b0:b0 + DMA_CHUNK, :])

            emit_products(0)
            if C > 1:
                emit_products(1)
            for i in range(C):
                m1, m2, m3 = mtiles.pop(i)
                npairs = C - i
                for jr in range(npairs):
                    if state["dps"] is None:
                        state["dps"] = psD.tile([65, PAIRS_PER_BANK, 42], f32,
                                                name="dps", tag="d")
                    dps = state["dps"]
                    slot = state["slot"]
                    for t, m in enumerate((m1, m2, m3)):
                        nc.tensor.matmul(dps[:, slot, :], m[:, jr, 0:65],
                                         smt[:, 42 * t:42 * t + 42],
                                         start=(t == 0), stop=(t == 2))
                    state["slot"] = slot + 1
                    if state["slot"] == PAIRS_PER_BANK:
                        flush_bank()
                if i + 2 < C:
                    emit_products(i + 2)
            flush_bank()
            flush_obank(state["ohalf"] * PAIRS_PER_BANK
                        + (state["slot"] or 0))

    nc.compile()
    return nc


_CACHE = {}


def _get_nc():
    if "nc" not in _CACHE:
        _CACHE["nc"] = build_nc()
    return _CACHE["nc"]


TRACE = False  # test harness can flip this to capture an NTFF profile


def kernel(x: np.ndarray) -> np.ndarray:
    from concourse.bass_utils import run_bass_kernel_spmd

    assert x.shape == (B, C, H, W) and x.dtype == np.float32
    nc = _get_nc()
    consts = _host_constants()
    in_maps = []
    for b in range(B):
        m = {"x": np.ascontiguousarray(x[b])}
        m.update(consts)
        in_maps.append(m)
    res = run_bass_kernel_spmd(nc, in_maps, core_ids=list(range(B)), trace=TRACE)
    _CACHE["last_results"] = res
    out = np.stack([r["out"] for r in res.results]).astype(np.float32)
    return out
